# revision 3
# baseline (speedup 1.0000x reference)
"""Per-pixel adaptive 5x5 conv (KPN) for Trainium2, 8-core data parallel.

out[g,h,w] = sum_{i,j} core[g,5i+j,h,w] * frames_pad[g,h+i-2,w+j-2]
with g = flattened (B,N) = 16 image planes; 2 planes per NeuronCore.

Host prep builds DMA-friendly layouts (all fp16):
  fin [2, 128, 5*2*4*518]: per partition p: [i-shift:5][parity:2][blk:4][518]
     fprep[img,p,i,par,blk,c] = Fpad[img, blk*128+p+i, (1-par)+c]
     parity copies keep every tap's 512-col slice 4-byte aligned so the
     DVE 2x fp16 mode engages for all 25 (i,j) taps.
  win [2, 5, 128, 5*4*512]: tap-group-major core weights
     wprep[img,tg,p,k,blk,c] = core[img, 5*tg+k, blk*128+p, c]
On chip per image: 1 frames DMA + 5 weight-group DMAs; 25 taps of
mul+add at FD=2048 (4 row-blocks fused per op), 19 taps on DVE (fp16
2x mode) and 6 on GpSimd; two accumulator chains merged at the end;
fp16->fp32 cast on the output DMA (SWDGE).
"""

import os
import sys

import numpy as np

for _p in ("/opt/trn_rl_repo",):
    if _p not in sys.path and os.path.isdir(_p):
        sys.path.insert(0, _p)

K = 5
NCORES = 8
IMGS_PER_CORE = 2
H = W = 512
NBLK = 4  # 128-row blocks per image
FCOLS = 518
F_FREE = K * 2 * NBLK * FCOLS  # 20720
W_FREE = K * NBLK * W  # 10240
O_FREE = NBLK * W  # 2048

# gpsimd gets ~1/4 of taps (it runs 2-input elemwise ~2.9x slower than
# DVE fp16-2x): one tap per 5-tap group + one extra.
GP_TAPS = frozenset({4, 9, 14, 19, 23, 24})

_compiled = {}
last_results = None  # BassKernelResults of the most recent run (for test.py)


def _build_nc():
    import concourse.bacc as bacc
    import concourse.mybir as mybir
    from concourse.tile import TileContext

    f16 = mybir.dt.float16
    f32 = mybir.dt.float32

    nc = bacc.Bacc(None, target_bir_lowering=False, debug=False)
    fin = nc.dram_tensor("fin", [IMGS_PER_CORE, 128, F_FREE], f16,
                         kind="ExternalInput")
    win = nc.dram_tensor("win", [IMGS_PER_CORE, K, 128, W_FREE], f16,
                         kind="ExternalInput")
    oout = nc.dram_tensor("oout", [IMGS_PER_CORE, 128, O_FREE], f32,
                          kind="ExternalOutput")

    with TileContext(nc) as tc:
        with (
            tc.tile_pool(name="fpool", bufs=2) as fpool,
            tc.tile_pool(name="wpool", bufs=3) as wpool,
            tc.tile_pool(name="apool", bufs=2) as apool,
            tc.tile_pool(name="tpool", bufs=4) as tpool,
        ):
            for img in range(IMGS_PER_CORE):
                f_t = fpool.tile([128, F_FREE], f16)
                nc.sync.dma_start(out=f_t[:], in_=fin[img])
                fv = f_t[:].rearrange("p (i par blk c) -> p i par blk c",
                                      i=K, par=2, blk=NBLK, c=FCOLS)

                acc_v = apool.tile([128, O_FREE], f16, tag="accv")
                acc_g = apool.tile([128, O_FREE], f16, tag="accg")
                av = acc_v[:].rearrange("p (blk c) -> p blk c", blk=NBLK)
                ag = acc_g[:].rearrange("p (blk c) -> p blk c", blk=NBLK)
                first = {id(av): True, id(ag): True}

                for tg in range(K):
                    w_t = wpool.tile([128, W_FREE], f16)
                    nc.sync.dma_start(out=w_t[:], in_=win[img, tg])
                    wv = w_t[:].rearrange("p (k blk c) -> p k blk c",
                                          k=K, blk=NBLK, c=W)
                    for k in range(K):
                        t = tg * K + k
                        i, j = divmod(t, K)
                        par = j & 1
                        joff = j + par
                        f_ap = fv[:, i, par, :, joff:joff + W]
                        w_ap = wv[:, k]
                        if t in GP_TAPS:
                            eng, acc, tag = nc.gpsimd, ag, "tmpg"
                        else:
                            eng, acc, tag = nc.vector, av, "tmpv"
                        if first[id(acc)]:
                            eng.tensor_mul(out=acc, in0=w_ap, in1=f_ap)
                            first[id(acc)] = False
                        else:
                            tmp = tpool.tile([128, O_FREE], f16, tag=tag)
                            tv = tmp[:].rearrange("p (blk c) -> p blk c",
                                                  blk=NBLK)
                            eng.tensor_mul(out=tv, in0=w_ap, in1=f_ap)
                            eng.tensor_add(out=acc, in0=acc, in1=tv)

                nc.vector.tensor_add(out=acc_v[:], in0=acc_v[:], in1=acc_g[:])
                # SWDGE cast fp16 -> fp32 on the store
                nc.gpsimd.dma_start(out=oout[img], in_=acc_v[:])
    nc.finalize()
    return nc


def _host_prep(frames, core):
    """Build per-core in_maps. frames [4,4,1,512,512] f32, core [4,4,25,1,512,512]."""
    G = NCORES * IMGS_PER_CORE  # 16
    F = np.ascontiguousarray(frames.reshape(G, H, W))
    Wc = core.reshape(G, K * K, H, W)

    # frames: pad rows 2/2, cols 3/4; Fp[g, r, c] = F[g, r-2, c-3]
    Fp = np.pad(F, ((0, 0), (2, 2), (3, 4))).astype(np.float16)
    fprep = np.empty((G, 128, K, 2, NBLK, FCOLS), np.float16)
    for i in range(K):
        for par in range(2):
            sl = Fp[:, i:i + H, (1 - par):(1 - par) + FCOLS]  # [G,512,518]
            fprep[:, :, i, par, :, :] = (
                sl.reshape(G, NBLK, 128, FCOLS).transpose(0, 2, 1, 3))

    # weights: [g][tg][p][k][blk][c]
    w16 = Wc.astype(np.float16)
    wprep = np.ascontiguousarray(
        w16.reshape(G, K, K, NBLK, 128, W).transpose(0, 1, 4, 2, 3, 5))

    in_maps = []
    for c in range(NCORES):
        g0 = c * IMGS_PER_CORE
        in_maps.append({
            "fin": np.ascontiguousarray(
                fprep[g0:g0 + IMGS_PER_CORE].reshape(IMGS_PER_CORE, 128, F_FREE)),
            "win": np.ascontiguousarray(
                wprep[g0:g0 + IMGS_PER_CORE].reshape(IMGS_PER_CORE, K, 128, W_FREE)),
        })
    return in_maps


def kernel(frames, core, bias):
    global last_results
    from concourse.bass_utils import run_bass_kernel_spmd

    frames = np.asarray(frames, dtype=np.float32)
    core = np.asarray(core, dtype=np.float32)

    if "nc" not in _compiled:
        _compiled["nc"] = _build_nc()
    nc = _compiled["nc"]

    in_maps = _host_prep(frames, core)
    trace = os.environ.get("KC_TRACE") == "1"
    res = run_bass_kernel_spmd(nc, in_maps, list(range(NCORES)), trace=trace)
    last_results = res

    G = NCORES * IMGS_PER_CORE
    out = np.empty((G, H, W), np.float32)
    for c in range(NCORES):
        o = res.results[c]["oout"]  # [2, 128, 2048] f32
        for img in range(IMGS_PER_CORE):
            out[c * IMGS_PER_CORE + img] = (
                o[img].reshape(128, NBLK, W).transpose(1, 0, 2).reshape(H, W))
    return out.reshape(4, 4, H, W)


# revision 4
# speedup vs baseline: 117.0484x; 117.0484x over previous
"""Per-pixel adaptive 5x5 conv (KPN) for Trainium2, 8-core data parallel.

out[g,h,w] = sum_{i,j} core[g,5i+j,h,w] * frames_pad[g,h+i-2,w+j-2]
with g = flattened (B,N) = 16 image planes; 2 planes per NeuronCore.

Host prep builds DMA-friendly layouts (all fp16):
  fin [2, 128, 5*2*4*518]: per partition p: [i-shift:5][parity:2][blk:4][518]
     fprep[img,p,i,par,blk,c] = Fpad[img, blk*128+p+i, (1-par)+c]
     parity copies keep every tap's 512-col slice 4-byte aligned so the
     DVE 2x fp16 mode engages for all 25 (i,j) taps.
  win [2, 5, 128, 5*4*512]: tap-group-major core weights
     wprep[img,tg,p,k,blk,c] = core[img, 5*tg+k, blk*128+p, c]
On chip per image: 1 frames DMA + 5 weight-group DMAs; 25 taps of
mul+add at FD=2048 (4 row-blocks fused per op), 19 taps on DVE (fp16
2x mode) and 6 on GpSimd; two accumulator chains merged at the end;
fp16->fp32 cast on the output DMA (SWDGE).
"""

import os
import sys

import numpy as np

for _p in ("/opt/trn_rl_repo",):
    if _p not in sys.path and os.path.isdir(_p):
        sys.path.insert(0, _p)

K = 5
NCORES = 8
IMGS_PER_CORE = 2
H = W = 512
NBLK = 4  # 128-row blocks per image
FCOLS = 518
F_FREE = K * 2 * NBLK * FCOLS  # 20720
W_FREE = K * NBLK * W  # 10240
O_FREE = NBLK * W  # 2048

# gpsimd gets ~1/4 of taps (it runs 2-input elemwise ~2.9x slower than
# DVE fp16-2x): one tap per 5-tap group + one extra.
GP_TAPS = frozenset({4, 9, 14, 19, 23, 24})

_compiled = {}
last_results = None  # BassKernelResults of the most recent run (for test.py)


def _build_nc():
    import concourse.bacc as bacc
    import concourse.mybir as mybir
    from concourse.tile import TileContext

    f16 = mybir.dt.float16
    f32 = mybir.dt.float32

    nc = bacc.Bacc(None, target_bir_lowering=False, debug=False)
    fin = nc.dram_tensor("fin", [IMGS_PER_CORE, 128, F_FREE], f16,
                         kind="ExternalInput")
    win = nc.dram_tensor("win", [IMGS_PER_CORE, K, 128, W_FREE], f16,
                         kind="ExternalInput")
    oout = nc.dram_tensor("oout", [IMGS_PER_CORE, 128, O_FREE], f32,
                          kind="ExternalOutput")

    with TileContext(nc) as tc:
        with (
            tc.tile_pool(name="fpool", bufs=2) as fpool,
            tc.tile_pool(name="wpool", bufs=3) as wpool,
            tc.tile_pool(name="apool", bufs=2) as apool,
            tc.tile_pool(name="tpool", bufs=4) as tpool,
        ):
            for img in range(IMGS_PER_CORE):
                f_t = fpool.tile([128, F_FREE], f16)
                nc.sync.dma_start(out=f_t[:], in_=fin[img])
                fv = f_t[:].rearrange("p (i par blk c) -> p i par blk c",
                                      i=K, par=2, blk=NBLK, c=FCOLS)

                acc_v = apool.tile([128, O_FREE], f16, tag="accv")
                acc_g = apool.tile([128, O_FREE], f16, tag="accg")
                av = acc_v[:].rearrange("p (blk c) -> p blk c", blk=NBLK)
                ag = acc_g[:].rearrange("p (blk c) -> p blk c", blk=NBLK)
                first = {id(av): True, id(ag): True}

                for tg in range(K):
                    w_t = wpool.tile([128, W_FREE], f16)
                    nc.sync.dma_start(out=w_t[:], in_=win[img, tg])
                    wv = w_t[:].rearrange("p (k blk c) -> p k blk c",
                                          k=K, blk=NBLK, c=W)
                    for k in range(K):
                        t = tg * K + k
                        i, j = divmod(t, K)
                        par = j & 1
                        joff = j + par
                        f_ap = fv[:, i, par, :, joff:joff + W]
                        w_ap = wv[:, k]
                        if t in GP_TAPS:
                            eng, acc, tag = nc.gpsimd, ag, "tmpg"
                        else:
                            eng, acc, tag = nc.vector, av, "tmpv"
                        if first[id(acc)]:
                            eng.tensor_mul(out=acc, in0=w_ap, in1=f_ap)
                            first[id(acc)] = False
                        else:
                            tmp = tpool.tile([128, O_FREE], f16, tag=tag)
                            tv = tmp[:].rearrange("p (blk c) -> p blk c",
                                                  blk=NBLK)
                            eng.tensor_mul(out=tv, in0=w_ap, in1=f_ap)
                            eng.tensor_add(out=acc, in0=acc, in1=tv)

                nc.vector.tensor_add(out=acc_v[:], in0=acc_v[:], in1=acc_g[:])
                # SWDGE cast fp16 -> fp32 on the store
                nc.gpsimd.dma_start(out=oout[img], in_=acc_v[:])
    nc.finalize()
    return nc


def _host_prep(frames, core):
    """Build per-core in_maps. frames [4,4,1,512,512] f32, core [4,4,25,1,512,512]."""
    G = NCORES * IMGS_PER_CORE  # 16
    F = np.ascontiguousarray(frames.reshape(G, H, W))
    Wc = core.reshape(G, K * K, H, W)

    # frames: pad rows 2/2, cols 3/4; Fp[g, r, c] = F[g, r-2, c-3]
    Fp = np.pad(F, ((0, 0), (2, 2), (3, 4))).astype(np.float16)
    fprep = np.empty((G, 128, K, 2, NBLK, FCOLS), np.float16)
    for i in range(K):
        for par in range(2):
            sl = Fp[:, i:i + H, (1 - par):(1 - par) + FCOLS]  # [G,512,518]
            fprep[:, :, i, par, :, :] = (
                sl.reshape(G, NBLK, 128, FCOLS).transpose(0, 2, 1, 3))

    # weights: [g][tg][p][k][blk][c]
    w16 = Wc.astype(np.float16)
    wprep = np.ascontiguousarray(
        w16.reshape(G, K, K, NBLK, 128, W).transpose(0, 1, 4, 2, 3, 5))

    in_maps = []
    for c in range(NCORES):
        g0 = c * IMGS_PER_CORE
        in_maps.append({
            "fin": np.ascontiguousarray(
                fprep[g0:g0 + IMGS_PER_CORE].reshape(IMGS_PER_CORE, 128, F_FREE)),
            "win": np.ascontiguousarray(
                wprep[g0:g0 + IMGS_PER_CORE].reshape(IMGS_PER_CORE, K, 128, W_FREE)),
        })
    return in_maps


def kernel(frames, core, bias):
    global last_results
    from concourse.bass_utils import run_bass_kernel_spmd

    frames = np.asarray(frames, dtype=np.float32)
    core = np.asarray(core, dtype=np.float32)

    if "nc" not in _compiled:
        _compiled["nc"] = _build_nc()
    nc = _compiled["nc"]

    in_maps = _host_prep(frames, core)
    trace = os.environ.get("KC_TRACE") == "1"
    tmpdir = os.environ.get("KC_TRACE_DIR") or None
    if tmpdir:
        os.makedirs(tmpdir, exist_ok=True)
    res = run_bass_kernel_spmd(nc, in_maps, list(range(NCORES)), trace=trace,
                               tmpdir=tmpdir)
    last_results = res

    G = NCORES * IMGS_PER_CORE
    out = np.empty((G, H, W), np.float32)
    for c in range(NCORES):
        o = res.results[c]["oout"]  # [2, 128, 2048] f32
        for img in range(IMGS_PER_CORE):
            out[c * IMGS_PER_CORE + img] = (
                o[img].reshape(128, NBLK, W).transpose(1, 0, 2).reshape(H, W))
    return out.reshape(4, 4, H, W)
